# revision 3
# baseline (speedup 1.0000x reference)
"""Local-strided block-sparse paged attention (decode) on 8 Trainium2 cores.

Strategy:
- Work unit = (sequence b, kv-head kv). The 4 q-heads sharing a kv head
  attend overlapping block sets (shared local window + per-phase stride
  columns), so each unit loads the UNION of its 4 heads' CSR rows once
  and computes all 4 heads against it (per-head bf16 0/1 masks applied
  after exp restore row membership + causality). This dedups ~2.5x of
  the K/V traffic and skips CSR padding entirely.
- Panels are converted to bf16 on host (tolerance is 2e-2; bf16 lands
  ~3.4e-3), halving HBM traffic again.
- 64 units are sorted by size and dealt rank-wise to 8 cores so every
  core gets 8 size-matched slots; one SPMD program (sizes baked per
  slot) serves all cores. The program is recompiled if the size
  signature changes (inputs are resolved on host every call).
- All K panels are packed into one mega DRAM tensor and all V+mask
  panels into another; each invocation issues exactly two large
  sequential DMAs on one HWDGE ring (K first, so QK compute starts
  while V is still streaming). Measured on HW this is the fastest
  arrangement: fragmenting transfers or adding ring/buffer concurrency
  delays the critical next-tile completion by 20-40%.
- Device per slot: chunked QK matmuls (scores[tok, 4 heads] in PSUM),
  exp -> mask-mult -> P (bf16), PV matmuls against V panels carrying an
  extra ones-column so the softmax denominator falls out of the same
  accumulation; per-slot normalize into one output tile, single store.
"""
import math
import numpy as np
import ml_dtypes

NCORES = 8
_PROG_CACHE: dict = {}


def _resolve_rows(layout_crow, layout_col, pbid, H, J=64):
    """Mirror the reference CSR row resolution (first-J trim + idx clip)."""
    W = layout_col.shape[1]
    rows = []
    for h in range(H):
        s = int(layout_crow[h, pbid])
        e = int(layout_crow[h, pbid + 1])
        n = min(max(e - s, 0), J)
        idx = np.clip(np.arange(s, s + n), 0, W - 1)
        rows.append(layout_col[h, idx].tolist())
    return rows


def _prepare(q, k_cache, v_cache, block_tables, context_lens, layout_crow, layout_col):
    B, H, D = q.shape
    KVH = k_cache.shape[1]
    BLK = v_cache.shape[3]
    X = k_cache.shape[4]
    G = H // KVH
    q_pid = context_lens.astype(np.int64) - 1
    pbid = q_pid // BLK

    bf16 = ml_dtypes.bfloat16

    # ---- build units: (b, kv) -> union block list + per-head validity ----
    units = []
    for b in range(B):
        rows_all = _resolve_rows(layout_crow, layout_col, int(pbid[b]), H)
        for kv in range(KVH):
            heads = [kv * G + j for j in range(G)]
            cnts = []
            for h in heads:
                c = {}
                for kb in rows_all[h]:
                    c[kb] = c.get(kb, 0) + 1
                cnts.append(c)
            mult = {}
            for c in cnts:
                for kb, n in c.items():
                    mult[kb] = max(mult.get(kb, 0), n)
            ulist = []
            copyidx = []
            for kb in sorted(mult):
                for i in range(mult[kb]):
                    ulist.append(kb)
                    copyidx.append(i)
            U = len(ulist)
            C = max(1, -(-U * BLK // 128))
            units.append(dict(b=b, kv=kv, heads=heads, ulist=ulist,
                              copyidx=copyidx, cnts=cnts, U=U, C=C))

    # ---- deal units to 8 cores x nslots slots, size-matched per slot ----
    nslots = -(-len(units) // NCORES)
    order = sorted(range(len(units)), key=lambda i: -units[i]["C"])
    slot_chunks = []
    assign = [[None] * nslots for _ in range(NCORES)]
    for k in range(nslots):
        grp = order[k * NCORES:(k + 1) * NCORES]
        slot_chunks.append(max(units[i]["C"] for i in grp))
        for c, i in enumerate(grp):
            assign[c][k] = i
    sig = tuple(slot_chunks) + (G, D)

    # ---- build per-core panels ----
    in_maps = []
    for c in range(NCORES):
        m = {}
        qq = np.zeros((D, G * nslots), bf16)
        for k in range(nslots):
            Ck = slot_chunks[k]
            T = Ck * 128
            kd = np.zeros((D, T), bf16)
            vt = np.zeros((128, Ck * 129), bf16)
            mt = np.zeros((128, Ck * G), bf16)
            ui = assign[c][k]
            if ui is not None:
                u = units[ui]
                b, kv, U = u["b"], u["kv"], u["U"]
                phys = block_tables[b, np.asarray(u["ulist"], np.int64)]
                # K: [U, D//X, BLK, X] -> [d = dx*X+xi, u*BLK+tok]
                kb = k_cache[phys, kv]
                kd[:, :U * BLK] = (
                    kb.transpose(1, 3, 0, 2).reshape(D, U * BLK).astype(bf16)
                )
                # V: [U, D, BLK] -> token-major [U*BLK, D], chunked + ones col
                vtok = np.zeros((T, D), np.float32)
                vtok[:U * BLK] = v_cache[phys, kv].transpose(0, 2, 1).reshape(U * BLK, D)
                varr = np.concatenate(
                    [vtok.reshape(Ck, 128, D),
                     np.ones((Ck, 128, 1), np.float32)], axis=2)
                vt[:] = varr.transpose(1, 0, 2).reshape(128, Ck * 129).astype(bf16)
                # mask [tok, head] -> [128, (chunk, head)]
                ul = np.asarray(u["ulist"], np.int64)
                ci = np.asarray(u["copyidx"], np.int64)
                pos = (ul[:, None] * BLK + np.arange(BLK)[None, :]).reshape(-1)
                mtok = np.zeros((T, G), np.float32)
                for j in range(G):
                    cnt = u["cnts"][j]
                    member = np.asarray(
                        [ci[i] < cnt.get(int(ul[i]), 0) for i in range(U)], bool)
                    ok = np.repeat(member, BLK) & (pos <= int(q_pid[b]))
                    mtok[:U * BLK, j] = ok.astype(np.float32)
                mt[:] = mtok.reshape(Ck, 128, G).transpose(1, 0, 2).reshape(
                    128, Ck * G).astype(bf16)
                qq[:, k * G:(k + 1) * G] = q[b, u["heads"]].T.astype(bf16)
            else:
                mt[:] = 1.0
            m[f"kd{k}"] = kd
            m[f"vm{k}"] = np.concatenate([vt, mt], axis=1)
        m["qq"] = qq
        # one mega K panel and one mega V+mask panel per core
        m["kg"] = np.concatenate(
            [m.pop(f"kd{k}") for k in range(nslots)], axis=1)
        m["vg"] = np.concatenate(
            [m.pop(f"vm{k}") for k in range(nslots)], axis=1)
        in_maps.append(m)
    return in_maps, assign, units, sig, nslots


def _build_program(sig, repeat=1, loop=0):
    import concourse.bacc as bacc
    import concourse.mybir as mybir
    from concourse.tile import TileContext

    slot_chunks = list(sig[:-2])
    G, D = sig[-2], sig[-1]
    nslots = len(slot_chunks)
    f32 = mybir.dt.float32
    bf16 = mybir.dt.bfloat16
    SM = 1.0 / math.sqrt(D)

    nc = bacc.Bacc("TRN2", target_bir_lowering=False)
    tot = sum(slot_chunks)
    kg = nc.dram_tensor("kg", [D, tot * 128], bf16, kind="ExternalInput")
    vg = nc.dram_tensor("vg", [128, tot * (129 + G)], bf16,
                        kind="ExternalInput")
    qq = nc.dram_tensor("qq", [D, G * nslots], bf16, kind="ExternalInput")
    out = nc.dram_tensor("out", [G, nslots * D], f32, kind="ExternalOutput")

    with TileContext(nc) as tc:
        with (
            tc.tile_pool(name="kv", bufs=3) as kvp,
            tc.tile_pool(name="small", bufs=4) as sp,
            tc.tile_pool(name="ps_sc", bufs=4, space="PSUM") as pp_sc,
            tc.tile_pool(name="ps_ov", bufs=4, space="PSUM") as pp_ov,
            tc.tile_pool(name="persist", bufs=2) as cp,
        ):
            def _one_body():
                qt = cp.tile([D, G * nslots], bf16, tag="qt")
                nc.sync.dma_start(out=qt[:], in_=qq[:])
                kgt = kvp.tile([D, tot * 128], bf16, tag="kg")
                nc.sync.dma_start(out=kgt[:], in_=kg[:])
                vgt = kvp.tile([128, tot * (129 + G)], bf16, tag="vg")
                nc.sync.dma_start(out=vgt[:], in_=vg[:])
                osb = cp.tile([G, nslots * D], f32, tag="osb")
                koff = 0
                voff = 0
                if True:
                  for k in range(nslots):
                    Ck = slot_chunks[k]
                    kt = kgt[:, koff:koff + Ck * 128]
                    koff += Ck * 128
                    vmt = vgt[:, voff:voff + Ck * (129 + G)]
                    voff += Ck * (129 + G)
                    vt = vmt[:, 0:Ck * 129]
                    mt = vmt[:, Ck * 129:Ck * (129 + G)]

                    sc = pp_sc.tile([128, Ck * G], f32, tag="sc")
                    for c in range(Ck):
                        nc.tensor.matmul(
                            sc[:, c * G:(c + 1) * G],
                            kt[:, c * 128:(c + 1) * 128],
                            qt[:, k * G:(k + 1) * G],
                            start=True, stop=True,
                        )
                    pe = sp.tile([128, Ck * G], bf16, tag="pe")
                    nc.scalar.activation(
                        pe[:], sc[:], mybir.ActivationFunctionType.Exp, scale=SM)
                    p = sp.tile([128, Ck * G], bf16, tag="p")
                    nc.vector.tensor_mul(p[:], pe[:], mt[:])
                    ov = pp_ov.tile([G, 129], f32, tag="ov")
                    for c in range(Ck):
                        nc.tensor.matmul(
                            ov[:], p[:, c * G:(c + 1) * G],
                            vt[:, c * 129:(c + 1) * 129],
                            start=(c == 0), stop=(c == Ck - 1),
                        )
                    rec = sp.tile([G, 1], f32, tag="rec")
                    nc.vector.reciprocal(rec[:], ov[:, 128:129])
                    nc.vector.tensor_scalar_mul(
                        osb[:, k * D:(k + 1) * D], ov[:, 0:128], rec[:])
                  # Store rides the Act HWDGE ring: on the SP ring it would
                  # FIFO-block the next iteration's K/V streams behind this
                  # iteration's compute tail (~5us idle per iter).
                  nc.scalar.dma_start(out=out[:], in_=osb[:])

            if loop:
                with tc.For_i(0, loop, 1,
                              hint_engines=(mybir.EngineType.PE,
                                            mybir.EngineType.DVE,
                                            mybir.EngineType.Activation)):
                    for _rep in range(repeat):
                        _one_body()
            else:
                for _rep in range(repeat):
                    _one_body()
    nc.compile()
    return nc


def _get_program(sig, repeat=1, loop=0):
    key = (sig, repeat, loop)
    nc = _PROG_CACHE.get(key)
    if nc is None:
        nc = _build_program(sig, repeat, loop)
        _PROG_CACHE[key] = nc
    return nc


def kernel(q, k_cache, v_cache, block_tables, context_lens, layout_crow, layout_col):
    from concourse.bass_utils import run_bass_kernel_spmd

    q = np.asarray(q, np.float32)
    k_cache = np.asarray(k_cache, np.float32)
    v_cache = np.asarray(v_cache, np.float32)
    block_tables = np.asarray(block_tables, np.int64)
    context_lens = np.asarray(context_lens, np.int64)
    layout_crow = np.asarray(layout_crow, np.int64)
    layout_col = np.asarray(layout_col, np.int64)

    B, H, D = q.shape
    KVH = k_cache.shape[1]
    G = H // KVH

    in_maps, assign, units, sig, nslots = _prepare(
        q, k_cache, v_cache, block_tables, context_lens, layout_crow, layout_col)

    nc = _get_program(sig)

    res = run_bass_kernel_spmd(nc, in_maps, core_ids=list(range(NCORES)))

    out = np.empty((B, H, D), np.float32)
    for c in range(NCORES):
        o = res.results[c]["out"]
        for k in range(nslots):
            ui = assign[c][k]
            if ui is None:
                continue
            u = units[ui]
            out[u["b"], u["heads"]] = o[:, k * D:(k + 1) * D]
    return out



# revision 6
# speedup vs baseline: 1.0222x; 1.0222x over previous
"""Local-strided block-sparse paged attention (decode) on 8 Trainium2 cores.

Strategy (v2 — 16-block-granular stream packing):
- Work unit = (sequence b, kv-head kv): the 4 q-heads sharing a kv head
  attend overlapping block sets, so each unit loads the UNION of its 4
  heads' CSR rows once; per-head 0/1 masks applied after exp restore row
  membership + causality. Panels are bf16 (tolerance 2e-2, bf16 ~3e-3).
- 64 units sorted by size, rank r = units[8r:8r+8] (consecutive-sorted
  is optimal for sum-of-rank-maxima); core c takes the c-th unit of each
  rank. Every core pads each unit to its rank max IN BLOCKS (16 tokens),
  not chunks, then concatenates the 8 ranks into ONE token stream cut
  into 128-token chunks — a chunk can straddle two ranks. This replaces
  v1's per-unit 128-rounding + chunk-matched slots (74 chunks/core) with
  block-granular matching (70 chunks/core), ~6% less HBM traffic.
- Per chunk the program computes scores for BOTH possibly-present ranks
  into 8 score columns (even-parity rank -> cols 0-3, odd -> 4-7); both
  matmuls share the chunk's stationary K panel. Garbage halves are
  masked to zero (masks are host-built), which also keeps every PSUM
  element written (exp of unwritten PSUM would poison with NaN).
- K panel is one mega DMA; V+ones+mask panels are interleaved per
  8-chunk group in a second mega DMA on the same SP HWDGE ring (K
  first so QK starts while V streams; group interleave lets PV(g)
  start as soon as group g lands). The output store rides the Act
  HWDGE ring — on the SP ring it would FIFO-block the next
  iteration's K/V streams behind this iteration's compute tail.
- PV accumulates per rank into a [4,129] PSUM tile across the rank's
  chunk segments; the V panels carry a ones-column so the softmax
  denominator falls out of the same accumulation; per-rank normalize
  into one output tile, single store.
"""
import math
import numpy as np
import ml_dtypes

NCORES = 8
NRANKS = 8
CG = 8          # chunks per score/exp/mask group
_PROG_CACHE: dict = {}


def _resolve_rows(layout_crow, layout_col, pbid, H, J=64):
    """Mirror the reference CSR row resolution (first-J trim + idx clip)."""
    W = layout_col.shape[1]
    rows = []
    for h in range(H):
        s = int(layout_crow[h, pbid])
        e = int(layout_crow[h, pbid + 1])
        n = min(max(e - s, 0), J)
        idx = np.clip(np.arange(s, s + n), 0, W - 1)
        rows.append(layout_col[h, idx].tolist())
    return rows


def _schedule(sig):
    """Static per-core schedule shared by host packing and program build.

    Returns (spans, NCH, groups): spans[r] is the token span of rank r
    (rank 7 extended over the final chunk pad), groups is a list of
    (first_chunk, n_chunks) pairs."""
    rank_blocks = list(sig[:-2])
    off = np.concatenate([[0], np.cumsum(rank_blocks)])
    TB = int(off[-1])
    NCH = -(-TB * 16 // 128)
    spans = []
    for r in range(NRANKS):
        t0, t1 = int(off[r]) * 16, int(off[r + 1]) * 16
        if r == NRANKS - 1:
            t1 = NCH * 128
        spans.append((t0, t1))
    groups = []
    c = 0
    while c < NCH:
        cg = min(CG, NCH - c)
        groups.append((c, cg))
        c += cg
    return spans, NCH, groups


def _prepare(q, k_cache, v_cache, block_tables, context_lens, layout_crow, layout_col):
    B, H, D = q.shape
    KVH = k_cache.shape[1]
    BLK = v_cache.shape[3]
    G = H // KVH
    q_pid = context_lens.astype(np.int64) - 1
    pbid = q_pid // BLK

    bf16 = ml_dtypes.bfloat16

    # ---- build units: (b, kv) -> union block list + per-head membership ----
    units = []
    for b in range(B):
        rows_all = _resolve_rows(layout_crow, layout_col, int(pbid[b]), H)
        for kv in range(KVH):
            heads = [kv * G + j for j in range(G)]
            cnts = []
            for h in heads:
                c = {}
                for kb in rows_all[h]:
                    c[kb] = c.get(kb, 0) + 1
                cnts.append(c)
            mult = {}
            for c in cnts:
                for kb, n in c.items():
                    mult[kb] = max(mult.get(kb, 0), n)
            ulist = []
            copyidx = []
            for kb in sorted(mult):
                for i in range(mult[kb]):
                    ulist.append(kb)
                    copyidx.append(i)
            units.append(dict(b=b, kv=kv, heads=heads, ulist=ulist,
                              copyidx=copyidx, cnts=cnts, U=len(ulist)))

    # ---- deal: sort desc, rank r = 8 consecutive units, one per core ----
    assert len(units) == NCORES * NRANKS
    order = sorted(range(len(units)), key=lambda i: -units[i]["U"])
    rank_blocks = []
    assign = [[None] * NRANKS for _ in range(NCORES)]
    for r in range(NRANKS):
        grp = order[r * NCORES:(r + 1) * NCORES]
        rank_blocks.append(max(units[i]["U"] for i in grp))
        for c, i in enumerate(grp):
            assign[c][r] = i
    sig = tuple(rank_blocks) + (G, D)
    spans, NCH, groups = _schedule(sig)
    NT = NCH * 128

    # ---- build per-core panels ----
    in_maps = []
    for c in range(NCORES):
        kd = np.zeros((D, NT), bf16)
        vtok = np.zeros((NT, D), np.float32)
        mtok = np.zeros((NT, 2 * G), np.float32)
        qq = np.zeros((D, NRANKS * G), bf16)
        for r in range(NRANKS):
            u = units[assign[c][r]]
            b, kv, U = u["b"], u["kv"], u["U"]
            t0 = spans[r][0]
            phys = block_tables[b, np.asarray(u["ulist"], np.int64)]
            kb = k_cache[phys, kv]          # [U, D//X, BLK, X]
            kd[:, t0:t0 + U * BLK] = (
                kb.transpose(1, 3, 0, 2).reshape(D, U * BLK).astype(bf16))
            vtok[t0:t0 + U * BLK] = (
                v_cache[phys, kv].transpose(0, 2, 1).reshape(U * BLK, D))
            ul = np.asarray(u["ulist"], np.int64)
            ci = np.asarray(u["copyidx"], np.int64)
            pos = (ul[:, None] * BLK + np.arange(BLK)[None, :]).reshape(-1)
            par = (r % 2) * G
            for j in range(G):
                cnt = u["cnts"][j]
                member = np.asarray(
                    [ci[i] < cnt.get(int(ul[i]), 0) for i in range(U)], bool)
                ok = np.repeat(member, BLK) & (pos <= int(q_pid[b]))
                mtok[t0:t0 + U * BLK, par + j] = ok.astype(np.float32)
            qq[:, r * G:(r + 1) * G] = q[b, u["heads"]].T.astype(bf16)

        # V+ones+mask interleaved per group: [cg*129 V cols | cg*2G mask]
        vparts = []
        for (g0, cg) in groups:
            va = vtok[g0 * 128:(g0 + cg) * 128].reshape(cg, 128, D)
            va = np.concatenate([va, np.ones((cg, 128, 1), np.float32)], 2)
            vparts.append(va.transpose(1, 0, 2).reshape(128, cg * 129))
            ma = mtok[g0 * 128:(g0 + cg) * 128].reshape(cg, 128, 2 * G)
            vparts.append(ma.transpose(1, 0, 2).reshape(128, cg * 2 * G))
        m = {"kg": kd,
             "vg": np.concatenate(vparts, axis=1).astype(bf16),
             "qq": qq}
        in_maps.append(m)
    return in_maps, assign, units, sig, NRANKS


def _build_program(sig, repeat=1, loop=0):
    import concourse.bacc as bacc
    import concourse.mybir as mybir
    from concourse.tile import TileContext

    G, D = sig[-2], sig[-1]
    spans, NCH, groups = _schedule(sig)
    NT = NCH * 128
    W8 = 2 * G                      # score cols per chunk
    VW = NCH * (129 + W8)           # vg panel cols
    f32 = mybir.dt.float32
    bf16 = mybir.dt.bfloat16
    SM = 1.0 / math.sqrt(D)

    # chunk -> (even-parity rank or fallback, odd-parity rank or fallback)
    chunk_ranks = []
    for c in range(NCH):
        lo, hi = c * 128, (c + 1) * 128
        pres = [r for r in range(NRANKS)
                if spans[r][0] < hi and spans[r][1] > lo]
        assert 1 <= len(pres) <= 2
        er = next((r for r in pres if r % 2 == 0), None)
        orr = next((r for r in pres if r % 2 == 1), None)
        if er is None:
            er = orr - 1 if orr > 0 else orr + 1
        if orr is None:
            orr = er + 1 if er < NRANKS - 1 else er - 1
        chunk_ranks.append((er, orr))
    rank_first_chunk = [spans[r][0] // 128 for r in range(NRANKS)]
    rank_last_chunk = [-(-spans[r][1] // 128) - 1 for r in range(NRANKS)]

    nc = bacc.Bacc("TRN2", target_bir_lowering=False)
    kg = nc.dram_tensor("kg", [D, NT], bf16, kind="ExternalInput")
    vg = nc.dram_tensor("vg", [128, VW], bf16, kind="ExternalInput")
    qq = nc.dram_tensor("qq", [D, NRANKS * G], bf16, kind="ExternalInput")
    out = nc.dram_tensor("out", [G, NRANKS * D], f32, kind="ExternalOutput")

    # vg col offset of each group's V block
    goff = []
    o = 0
    for (g0, cg) in groups:
        goff.append(o)
        o += cg * (129 + W8)

    with TileContext(nc) as tc:
        with (
            tc.tile_pool(name="kv", bufs=2) as kvp,
            tc.tile_pool(name="small", bufs=4) as sp,
            tc.tile_pool(name="ps_sc", bufs=4, space="PSUM") as pp_sc,
            tc.tile_pool(name="ps_ov", bufs=4, space="PSUM") as pp_ov,
            tc.tile_pool(name="persist", bufs=2) as cp,
        ):
            def _one_body():
                qt = cp.tile([D, NRANKS * G], bf16, tag="qt")
                nc.sync.dma_start(out=qt[:], in_=qq[:])
                kgt = kvp.tile([D, NT], bf16, tag="kg")
                nc.sync.dma_start(out=kgt[:], in_=kg[:])
                vgt = kvp.tile([128, VW], bf16, tag="vg")
                nc.sync.dma_start(out=vgt[:], in_=vg[:])
                osb = cp.tile([G, NRANKS * D], f32, tag="osb")
                ov = [None] * NRANKS
                for gi, (g0, cg) in enumerate(groups):
                    sc = pp_sc.tile([128, cg * W8], f32, tag="sc")
                    for ci in range(cg):
                        c = g0 + ci
                        kt = kgt[:, c * 128:(c + 1) * 128]
                        er, orr = chunk_ranks[c]
                        nc.tensor.matmul(
                            sc[:, ci * W8:ci * W8 + G], kt,
                            qt[:, er * G:(er + 1) * G],
                            start=True, stop=True)
                        nc.tensor.matmul(
                            sc[:, ci * W8 + G:ci * W8 + W8], kt,
                            qt[:, orr * G:(orr + 1) * G],
                            start=True, stop=True)
                    pe = sp.tile([128, cg * W8], bf16, tag="pe")
                    nc.scalar.activation(
                        pe[:], sc[:], mybir.ActivationFunctionType.Exp,
                        scale=SM)
                    p = sp.tile([128, cg * W8], bf16, tag="p")
                    mt = vgt[:, goff[gi] + cg * 129:goff[gi] + cg * (129 + W8)]
                    nc.vector.tensor_mul(p[:], pe[:], mt)
                    for ci in range(cg):
                        c = g0 + ci
                        vt = vgt[:, goff[gi] + ci * 129:goff[gi] + (ci + 1) * 129]
                        for r in sorted(set(chunk_ranks[c])):
                            t0, t1 = spans[r]
                            lo = max(t0, c * 128) - c * 128
                            hi = min(t1, (c + 1) * 128) - c * 128
                            if hi <= lo:
                                continue
                            par = (r % 2) * G
                            if ov[r] is None:
                                ov[r] = pp_ov.tile(
                                    [G, 129], f32, tag="ov", name=f"ov{r}")
                            # Full-chunk rows: base_partition must be
                            # 0/32/64, and rows outside the rank's span
                            # have p==0 in its parity columns (host
                            # masks), so they contribute exactly zero.
                            nc.tensor.matmul(
                                ov[r][:],
                                p[:, ci * W8 + par:ci * W8 + par + G],
                                vt[:, :],
                                start=(c == rank_first_chunk[r]),
                                stop=(c == rank_last_chunk[r]))
                            if c == rank_last_chunk[r]:
                                rec = sp.tile([G, 1], f32, tag="rec")
                                nc.vector.reciprocal(rec[:], ov[r][:, 128:129])
                                nc.vector.tensor_scalar_mul(
                                    osb[:, r * D:(r + 1) * D],
                                    ov[r][:, 0:128], rec[:])
                nc.scalar.dma_start(out=out[:], in_=osb[:])

            if loop:
                with tc.For_i(0, loop, 1,
                              hint_engines=(mybir.EngineType.PE,
                                            mybir.EngineType.DVE,
                                            mybir.EngineType.Activation)):
                    for _rep in range(repeat):
                        _one_body()
            else:
                for _rep in range(repeat):
                    _one_body()
    nc.compile()
    return nc


def _get_program(sig, repeat=1, loop=0):
    key = (sig, repeat, loop)
    nc = _PROG_CACHE.get(key)
    if nc is None:
        nc = _build_program(sig, repeat, loop)
        _PROG_CACHE[key] = nc
    return nc


def kernel(q, k_cache, v_cache, block_tables, context_lens, layout_crow, layout_col):
    from concourse.bass_utils import run_bass_kernel_spmd

    q = np.asarray(q, np.float32)
    k_cache = np.asarray(k_cache, np.float32)
    v_cache = np.asarray(v_cache, np.float32)
    block_tables = np.asarray(block_tables, np.int64)
    context_lens = np.asarray(context_lens, np.int64)
    layout_crow = np.asarray(layout_crow, np.int64)
    layout_col = np.asarray(layout_col, np.int64)

    B, H, D = q.shape

    in_maps, assign, units, sig, nranks = _prepare(
        q, k_cache, v_cache, block_tables, context_lens, layout_crow, layout_col)

    nc = _get_program(sig)

    res = run_bass_kernel_spmd(nc, in_maps, core_ids=list(range(NCORES)))

    out = np.empty((B, H, D), np.float32)
    for c in range(NCORES):
        o = res.results[c]["out"]
        for r in range(nranks):
            u = units[assign[c][r]]
            out[u["b"], u["heads"]] = o[:, r * D:(r + 1) * D]
    return out


# revision 7
# speedup vs baseline: 1.0280x; 1.0056x over previous
"""Local-strided block-sparse paged attention (decode) on 8 Trainium2 cores.

Strategy (v2 — 16-block-granular stream packing):
- Work unit = (sequence b, kv-head kv): the 4 q-heads sharing a kv head
  attend overlapping block sets, so each unit loads the UNION of its 4
  heads' CSR rows once; per-head 0/1 masks applied after exp restore row
  membership + causality. Panels are bf16 (tolerance 2e-2, bf16 ~3e-3).
- 64 units sorted by size, rank r = units[8r:8r+8] (consecutive-sorted
  is optimal for sum-of-rank-maxima); core c takes the c-th unit of each
  rank. Every core pads each unit to its rank max IN BLOCKS (16 tokens),
  not chunks, then concatenates the 8 ranks into ONE token stream cut
  into 128-token chunks — a chunk can straddle two ranks. This replaces
  v1's per-unit 128-rounding + chunk-matched slots (74 chunks/core) with
  block-granular matching (70 chunks/core), ~6% less HBM traffic.
- Per chunk the program computes scores for BOTH possibly-present ranks
  into 8 score columns (even-parity rank -> cols 0-3, odd -> 4-7); both
  matmuls share the chunk's stationary K panel. Garbage halves are
  masked to zero (masks are host-built), which also keeps every PSUM
  element written (exp of unwritten PSUM would poison with NaN).
- K panel is one mega DMA; V+ones+mask panels are interleaved per
  8-chunk group in a second mega DMA on the same SP HWDGE ring (K
  first so QK starts while V streams; group interleave lets PV(g)
  start as soon as group g lands). The output store rides the Act
  HWDGE ring — on the SP ring it would FIFO-block the next
  iteration's K/V streams behind this iteration's compute tail.
- PV accumulates per rank into a [4,129] PSUM tile across the rank's
  chunk segments; the V panels carry a ones-column so the softmax
  denominator falls out of the same accumulation; per-rank normalize
  into one output tile, single store.
"""
import math
import numpy as np
import ml_dtypes

NCORES = 8
NRANKS = 8
CG = 8          # chunks per score/exp/mask group
_PROG_CACHE: dict = {}


def _resolve_rows(layout_crow, layout_col, pbid, H, J=64):
    """Mirror the reference CSR row resolution (first-J trim + idx clip)."""
    W = layout_col.shape[1]
    rows = []
    for h in range(H):
        s = int(layout_crow[h, pbid])
        e = int(layout_crow[h, pbid + 1])
        n = min(max(e - s, 0), J)
        idx = np.clip(np.arange(s, s + n), 0, W - 1)
        rows.append(layout_col[h, idx].tolist())
    return rows


def _schedule(sig):
    """Static per-core schedule shared by host packing and program build.

    Returns (spans, NCH, groups): spans[r] is the token span of rank r
    (rank 7 extended over the final chunk pad), groups is a list of
    (first_chunk, n_chunks) pairs."""
    rank_blocks = list(sig[:-2])
    off = np.concatenate([[0], np.cumsum(rank_blocks)])
    TB = int(off[-1])
    NCH = -(-TB * 16 // 128)
    spans = []
    for r in range(NRANKS):
        t0, t1 = int(off[r]) * 16, int(off[r + 1]) * 16
        if r == NRANKS - 1:
            t1 = NCH * 128
        spans.append((t0, t1))
    groups = []
    c = 0
    while c < NCH:
        cg = min(CG, NCH - c)
        groups.append((c, cg))
        c += cg
    return spans, NCH, groups


def _prepare(q, k_cache, v_cache, block_tables, context_lens, layout_crow, layout_col):
    B, H, D = q.shape
    KVH = k_cache.shape[1]
    BLK = v_cache.shape[3]
    G = H // KVH
    q_pid = context_lens.astype(np.int64) - 1
    pbid = q_pid // BLK

    bf16 = ml_dtypes.bfloat16

    # ---- build units: (b, kv) -> union block list + per-head membership ----
    units = []
    for b in range(B):
        rows_all = _resolve_rows(layout_crow, layout_col, int(pbid[b]), H)
        for kv in range(KVH):
            heads = [kv * G + j for j in range(G)]
            cnts = []
            for h in heads:
                c = {}
                for kb in rows_all[h]:
                    c[kb] = c.get(kb, 0) + 1
                cnts.append(c)
            mult = {}
            for c in cnts:
                for kb, n in c.items():
                    mult[kb] = max(mult.get(kb, 0), n)
            ulist = []
            copyidx = []
            for kb in sorted(mult):
                for i in range(mult[kb]):
                    ulist.append(kb)
                    copyidx.append(i)
            units.append(dict(b=b, kv=kv, heads=heads, ulist=ulist,
                              copyidx=copyidx, cnts=cnts, U=len(ulist)))

    # ---- deal: sort desc, rank r = 8 consecutive units, one per core ----
    assert len(units) == NCORES * NRANKS
    order = sorted(range(len(units)), key=lambda i: -units[i]["U"])
    rank_blocks = []
    assign = [[None] * NRANKS for _ in range(NCORES)]
    for r in range(NRANKS):
        grp = order[r * NCORES:(r + 1) * NCORES]
        rank_blocks.append(max(units[i]["U"] for i in grp))
        for c, i in enumerate(grp):
            assign[c][r] = i
    sig = tuple(rank_blocks) + (G, D)
    spans, NCH, groups = _schedule(sig)
    NT = NCH * 128

    # ---- build per-core panels ----
    in_maps = []
    for c in range(NCORES):
        kd = np.zeros((D, NT), bf16)
        vtok = np.zeros((NT, D), np.float32)
        mtok = np.zeros((NT, 2 * G), np.float32)
        qq = np.zeros((D, NRANKS * G), bf16)
        for r in range(NRANKS):
            u = units[assign[c][r]]
            b, kv, U = u["b"], u["kv"], u["U"]
            t0 = spans[r][0]
            phys = block_tables[b, np.asarray(u["ulist"], np.int64)]
            kb = k_cache[phys, kv]          # [U, D//X, BLK, X]
            kd[:, t0:t0 + U * BLK] = (
                kb.transpose(1, 3, 0, 2).reshape(D, U * BLK).astype(bf16))
            vtok[t0:t0 + U * BLK] = (
                v_cache[phys, kv].transpose(0, 2, 1).reshape(U * BLK, D))
            ul = np.asarray(u["ulist"], np.int64)
            ci = np.asarray(u["copyidx"], np.int64)
            pos = (ul[:, None] * BLK + np.arange(BLK)[None, :]).reshape(-1)
            par = (r % 2) * G
            for j in range(G):
                cnt = u["cnts"][j]
                member = np.asarray(
                    [ci[i] < cnt.get(int(ul[i]), 0) for i in range(U)], bool)
                ok = np.repeat(member, BLK) & (pos <= int(q_pid[b]))
                mtok[t0:t0 + U * BLK, par + j] = ok.astype(np.float32)
            qq[:, r * G:(r + 1) * G] = q[b, u["heads"]].T.astype(bf16)

        # V+ones+mask interleaved per group: [cg*129 V cols | cg*2G mask]
        vparts = []
        for (g0, cg) in groups:
            va = vtok[g0 * 128:(g0 + cg) * 128].reshape(cg, 128, D)
            va = np.concatenate([va, np.ones((cg, 128, 1), np.float32)], 2)
            vparts.append(va.transpose(1, 0, 2).reshape(128, cg * 129))
            ma = mtok[g0 * 128:(g0 + cg) * 128].reshape(cg, 128, 2 * G)
            vparts.append(ma.transpose(1, 0, 2).reshape(128, cg * 2 * G))
        m = {"kg": kd,
             "vg": np.concatenate(vparts, axis=1).astype(bf16),
             "qq": qq}
        in_maps.append(m)
    return in_maps, assign, units, sig, NRANKS


def _build_program(sig, repeat=1, loop=0):
    import concourse.bacc as bacc
    import concourse.mybir as mybir
    from concourse.tile import TileContext

    G, D = sig[-2], sig[-1]
    spans, NCH, groups = _schedule(sig)
    NT = NCH * 128
    W8 = 2 * G                      # score cols per chunk
    VW = NCH * (129 + W8)           # vg panel cols
    f32 = mybir.dt.float32
    bf16 = mybir.dt.bfloat16
    SM = 1.0 / math.sqrt(D)

    # chunk -> (even-parity rank or fallback, odd-parity rank or fallback)
    chunk_ranks = []
    for c in range(NCH):
        lo, hi = c * 128, (c + 1) * 128
        pres = [r for r in range(NRANKS)
                if spans[r][0] < hi and spans[r][1] > lo]
        assert 1 <= len(pres) <= 2
        er = next((r for r in pres if r % 2 == 0), None)
        orr = next((r for r in pres if r % 2 == 1), None)
        if er is None:
            er = orr - 1 if orr > 0 else orr + 1
        if orr is None:
            orr = er + 1 if er < NRANKS - 1 else er - 1
        chunk_ranks.append((er, orr))
    rank_first_chunk = [spans[r][0] // 128 for r in range(NRANKS)]
    rank_last_chunk = [-(-spans[r][1] // 128) - 1 for r in range(NRANKS)]

    nc = bacc.Bacc("TRN2", target_bir_lowering=False)
    kg = nc.dram_tensor("kg", [D, NT], bf16, kind="ExternalInput")
    vg = nc.dram_tensor("vg", [128, VW], bf16, kind="ExternalInput")
    qq = nc.dram_tensor("qq", [D, NRANKS * G], bf16, kind="ExternalInput")
    out = nc.dram_tensor("out", [G, NRANKS * D], f32, kind="ExternalOutput")

    # vg col offset of each group's V block
    goff = []
    o = 0
    for (g0, cg) in groups:
        goff.append(o)
        o += cg * (129 + W8)

    with TileContext(nc) as tc:
        with (
            tc.tile_pool(name="kv", bufs=2) as kvp,
            tc.tile_pool(name="small", bufs=4) as sp,
            tc.tile_pool(name="ps_sc", bufs=4, space="PSUM") as pp_sc,
            tc.tile_pool(name="ps_ov", bufs=4, space="PSUM") as pp_ov,
            tc.tile_pool(name="persist", bufs=2) as cp,
        ):
            def _one_body():
                qt = cp.tile([D, NRANKS * G], bf16, tag="qt")
                nc.sync.dma_start(out=qt[:], in_=qq[:])
                kgt = kvp.tile([D, NT], bf16, tag="kg")
                nc.sync.dma_start(out=kgt[:], in_=kg[:])
                vgt = kvp.tile([128, VW], bf16, tag="vg")
                nc.sync.dma_start(out=vgt[:], in_=vg[:])
                osb = cp.tile([G, NRANKS * D], f32, tag="osb")
                ov = [None] * NRANKS

                def _qk(gi):
                    g0, cg = groups[gi]
                    sc = pp_sc.tile([128, cg * W8], f32, tag="sc", name="sc")
                    for ci in range(cg):
                        c = g0 + ci
                        kt = kgt[:, c * 128:(c + 1) * 128]
                        er, orr = chunk_ranks[c]
                        nc.tensor.matmul(
                            sc[:, ci * W8:ci * W8 + G], kt,
                            qt[:, er * G:(er + 1) * G],
                            start=True, stop=True)
                        nc.tensor.matmul(
                            sc[:, ci * W8 + G:ci * W8 + W8], kt,
                            qt[:, orr * G:(orr + 1) * G],
                            start=True, stop=True)
                    pe = sp.tile([128, cg * W8], bf16, tag="pe", name="pe")
                    nc.scalar.activation(
                        pe[:], sc[:], mybir.ActivationFunctionType.Exp,
                        scale=SM)
                    p = sp.tile([128, cg * W8], bf16, tag="p", name="p")
                    mt = vgt[:, goff[gi] + cg * 129:goff[gi] + cg * (129 + W8)]
                    nc.vector.tensor_mul(p[:], pe[:], mt)
                    return p

                def _pv(gi, p):
                    g0, cg = groups[gi]
                    for ci in range(cg):
                        c = g0 + ci
                        vt = vgt[:, goff[gi] + ci * 129:goff[gi] + (ci + 1) * 129]
                        for r in sorted(set(chunk_ranks[c])):
                            t0, t1 = spans[r]
                            lo = max(t0, c * 128) - c * 128
                            hi = min(t1, (c + 1) * 128) - c * 128
                            if hi <= lo:
                                continue
                            par = (r % 2) * G
                            if ov[r] is None:
                                ov[r] = pp_ov.tile(
                                    [G, 129], f32, tag="ov", name=f"ov{r}")
                            # Full-chunk rows: base_partition must be
                            # 0/32/64, and rows outside the rank's span
                            # have p==0 in its parity columns (host
                            # masks), so they contribute exactly zero.
                            nc.tensor.matmul(
                                ov[r][:],
                                p[:, ci * W8 + par:ci * W8 + par + G],
                                vt[:, :],
                                start=(c == rank_first_chunk[r]),
                                stop=(c == rank_last_chunk[r]))
                            if c == rank_last_chunk[r]:
                                rec = sp.tile([G, 1], f32, tag="rec",
                                              name="rec")
                                nc.vector.reciprocal(rec[:], ov[r][:, 128:129])
                                nc.vector.tensor_scalar_mul(
                                    osb[:, r * D:(r + 1) * D],
                                    ov[r][:, 0:128], rec[:])

                # Software-pipeline: emit QK(g+1) before PV(g) so the
                # in-order PE never idles waiting on group g's exp/mask
                # chain — it streams the next group's scores meanwhile.
                pprev = _qk(0)
                for gi in range(1, len(groups)):
                    pcur = _qk(gi)
                    _pv(gi - 1, pprev)
                    pprev = pcur
                _pv(len(groups) - 1, pprev)
                nc.scalar.dma_start(out=out[:], in_=osb[:])

            if loop:
                with tc.For_i(0, loop, 1,
                              hint_engines=(mybir.EngineType.PE,
                                            mybir.EngineType.DVE,
                                            mybir.EngineType.Activation)):
                    for _rep in range(repeat):
                        _one_body()
            else:
                for _rep in range(repeat):
                    _one_body()
    nc.compile()
    return nc


def _get_program(sig, repeat=1, loop=0):
    key = (sig, repeat, loop)
    nc = _PROG_CACHE.get(key)
    if nc is None:
        nc = _build_program(sig, repeat, loop)
        _PROG_CACHE[key] = nc
    return nc


def kernel(q, k_cache, v_cache, block_tables, context_lens, layout_crow, layout_col):
    from concourse.bass_utils import run_bass_kernel_spmd

    q = np.asarray(q, np.float32)
    k_cache = np.asarray(k_cache, np.float32)
    v_cache = np.asarray(v_cache, np.float32)
    block_tables = np.asarray(block_tables, np.int64)
    context_lens = np.asarray(context_lens, np.int64)
    layout_crow = np.asarray(layout_crow, np.int64)
    layout_col = np.asarray(layout_col, np.int64)

    B, H, D = q.shape

    in_maps, assign, units, sig, nranks = _prepare(
        q, k_cache, v_cache, block_tables, context_lens, layout_crow, layout_col)

    nc = _get_program(sig)

    res = run_bass_kernel_spmd(nc, in_maps, core_ids=list(range(NCORES)))

    out = np.empty((B, H, D), np.float32)
    for c in range(NCORES):
        o = res.results[c]["out"]
        for r in range(nranks):
            u = units[assign[c][r]]
            out[u["b"], u["heads"]] = o[:, r * D:(r + 1) * D]
    return out
